# revision 57
# baseline (speedup 1.0000x reference)
"""Trainium2 Bass kernel for nn_Graph_Critic_Model (gnn_message_passing).

Math (with the problem's fixed self-loop edge_index, the GCNConv collapses):
  X  = relu(obs @ W1 + b1)
  Xg = relu(X @ Wg + bg)                    # GCN with deg=2 self-loops == plain linear
  mu, sd = global mean/std over all Xg elements
  Xn = (Xg - mu)/(sd+eps) * ln_w + ln_b
  gate = sigmoid(Xn @ Wgate + bgate); pooled = sum(gate * Xn, axis=0)
  value = MLP(pooled); out = value * mask

Device layout: hid-major (features on partitions, nodes on free dim).
Data-parallel over nodes across 8 cores; two tiny AllReduces (LN stats, pooled).

v2 structure (per core, 16384 nodes, 32 chunks of 512):
  - weights packed into 2 DMAs; obs stages prefetched first (fast start)
  - phase A software-pipelined on the PE: mm2(c), mm1(c+1), gate(c-1) with
    ps_x bufs=4 / ps_xg bufs=3 so the PE streams continuously (p-state ramp)
  - Xg stored bf16 in two resident [128,16384] tiles
  - gate matmul lhsT = [wgln | ones] so the per-node column-sum (for the LN
    mean) rides along free; sumsq via all-bf16 STT (4x DVE mode)
  - phase C: sigmoid -> [1,16384] row -> stride-0-AP DMA broadcast to
    [128,2048] blocks -> all-bf16 STT with accum for the gated pooling
  - MLP entirely column-major (no transposes)
"""
import re
import numpy as np

N_TOTAL = 131072
F_DIM = 128
HID = 256
POL = 512
NCORES = 8
EPS = 1e-5
N_SH = N_TOTAL // NCORES
CH = 512          # nodes per compute chunk (one PSUM bank)
STAGE = 2048      # obs staging width
BLK = 2048        # phase C pooling block


def _split_excess_waits(nc, maxw=1):
    """walrus here rejects instructions with more than ~2 sem waits. Hoist
    excess waits onto dedicated nops placed just before the instruction on the
    same engine queue (waits are cumulative thresholds, so this is
    semantics-preserving)."""
    import concourse.mybir as mybir

    for blk in nc.m.functions[0].blocks:
        out = []
        changed = False
        for inst in blk.instructions:
            si = inst.sync_info
            if si is not None and len(si.on_wait) > maxw:
                waits = list(si.on_wait)
                extra, keep = waits[:-maxw], waits[-maxw:]
                for j in range(0, len(extra), maxw):
                    nop = mybir.InstNoOp(
                        name=f"{inst.name}.wsplit{j}",
                        sync_info=mybir.SyncInfo(on_wait=extra[j:j + maxw],
                                                 on_update=[]),
                        bass_nofuse=True,
                        engine=inst.engine,
                    )
                    nc.register_instruction(nop, overwrite=True)
                    out.append(nop)
                inst.sync_info = mybir.SyncInfo(
                    on_wait=keep, on_update=list(si.on_update))
                changed = True
            out.append(inst)
        if changed:
            blk.instructions = out


def _apply_tile_patch():
    """TileContext's tail drain collects one wait per logical proc on a single
    Drain instruction; split into one nop per proc before a clean drain, then
    run the global excess-wait splitter over the whole module."""
    from concourse.tile import TileContext
    from concourse.vector_clock import ScopedClock, VectorClock

    def _drain_and_barrier_split(self, tick_clock, wait_clock):
        gc = tick_clock.global_clock
        vals = [int(x) for x in re.findall(r"\d+", str(gc))]
        n = len(vals)
        for i, v in enumerate(vals):
            if v > 0:
                nop = self.nc.sync.nop(nofuse=True)
                vc = VectorClock([v if j == i else 0 for j in range(n)])
                wait_clock.add_sem_waits(nop.ins, ScopedClock({None: vc}))
        self.nc.sync.drain()
        self.nc.all_engine_barrier()
        popped = self.nc._tile_sem_poison_stack.pop()
        assert popped is self._sem_poison
        self.nc.clear_and_free_semaphores(list(self.sems.allocated().values()))
        self.nc.all_engine_barrier()
        _split_excess_waits(self.nc)

    TileContext._drain_and_barrier = _drain_and_barrier_split


def build(n_sh=N_SH, ncores=NCORES, total_nodes=N_TOTAL, dbg=False):
    import concourse.bass as bass
    import concourse.mybir as mybir
    import concourse.tile as tile

    _apply_tile_patch()

    f32 = mybir.dt.float32
    fr = mybir.dt.float32r
    bf16 = mybir.dt.bfloat16
    AF = mybir.ActivationFunctionType
    OP = mybir.AluOpType
    AX = mybir.AxisListType

    n_chunks = n_sh // CH          # 32
    n_stages = n_sh // STAGE       # 8
    cps = STAGE // CH              # 4 chunks per stage
    n_blk = n_sh // BLK            # 8
    ncols = n_sh // 128            # 128
    MTOT = float(total_nodes * HID)
    rg = [list(range(ncores))]

    # packed weight layouts: fr pack (phase A matmuls + row-MLP moving
    # blocks), tiny f32 pack (Wv: fp32r rejects moving free dim 1),
    # fr bias-row pack [1, 1280]
    W1_O = 0                       # [128, 256]
    WG_O = 256                     # 2 x [128, 256]
    WD_O = WG_O + 512              # 2 x [128, 256]
    WP1_O = WD_O + 512             # 2 x [128, 512]
    WP2_O = WP1_O + 1024           # 4 x [128, 512]
    WCOLS = WP2_O + 2048           # 4352
    BD_O = 0                       # [1, 256]
    BP1_O = 256                    # [1, 512]
    BP2_O = 768                    # [1, 512]
    BRCOLS = 1280

    nc = bass.Bass()
    dp = nc.declare_dram_parameter
    obsTd = dp("obsT", [F_DIM, n_sh], fr, isOutput=False)
    maskvd = dp("maskv", [128, ncols], f32, isOutput=False)
    wpackd = dp("wpack", [128, WCOLS], fr, isOutput=False)
    wvd = dp("wv", [128, 4], f32, isOutput=False)
    browsd = dp("brows", [1, BRCOLS], fr, isOutput=False)
    cpackd = dp("cpack", [128, 26], f32, isOutput=False)
    outd = dp("out", [128, ncols], f32, isOutput=True)
    bf16_ = mybir.dt.bfloat16
    if dbg:
        dbg_xg0 = dp("dbg_xg0", [128, 512], bf16_, isOutput=True)
        dbg_glr2 = dp("dbg_glr2", [2, n_sh], bf16_, isOutput=True)
        dbg_glx = dp("dbg_glx", [n_sh // CH, CH], bf16_, isOutput=True)
        dbg_gsx = dp("dbg_gsx", [n_sh // CH, CH], bf16_, isOutput=True)
        dbg_stats = dp("dbg_stats", [1, 2], f32, isOutput=True)
        dbg_statsg = dp("dbg_statsg", [1, 2], f32, isOutput=True)
        dbg_gate = dp("dbg_gate", [n_sh // CH, CH], bf16_, isOutput=True)
        dbg_pack = dp("dbg_pack", [128, 3], f32, isOutput=True)
        dbg_arp = dp("dbg_arp", [128, 3], f32, isOutput=True)
        dbg_poolc = dp("dbg_poolc", [128, 2], fr, isOutput=True)
        dbg_h1 = dp("dbg_h1", [128, 2], fr, isOutput=True)
        dbg_grep = dp("dbg_grep", [128, BLK], bf16_, isOutput=True)
        dbg_obs0 = dp("dbg_obs0", [128, 512], fr, isOutput=True)
        dbg_px0 = dp("dbg_px0", [128, 512], f32, isOutput=True)
        dbg_b1t = dp("dbg_b1t", [128, 2], f32, isOutput=True)
        dbg_csb = dp("dbg_csb", [128, 26], f32, isOutput=True)
        dbg_xt0 = dp("dbg_xt0", [128, 512], fr, isOutput=True)
        dbg_xt1 = dp("dbg_xt1", [128, 512], fr, isOutput=True)

    with tile.TileContext(nc) as tc:
        with tc.tile_pool(name="const", bufs=1) as const, \
             tc.tile_pool(name="stage", bufs=2) as stage_p, \
             tc.tile_pool(name="xt", bufs=4) as xt_p, \
             tc.tile_pool(name="scrq", bufs=2) as scrq_p, \
             tc.tile_pool(name="scrp", bufs=4) as scrp_p, \
             tc.tile_pool(name="grep", bufs=4) as grep_p, \
             tc.tile_pool(name="sm", bufs=1) as sm_p, \
             tc.tile_pool(name="psx", bufs=3, space="PSUM") as ps_x, \
             tc.tile_pool(name="psxg", bufs=3, space="PSUM") as ps_xg, \
             tc.tile_pool(name="psgl", bufs=1, space="PSUM") as ps_gl, \
             tc.tile_pool(name="dram", bufs=1, space="DRAM") as dram:

            # ---- obs stages first so compute starts early ----
            stage_tiles = {}

            def issue_stage(s):
                t = stage_p.tile([128, STAGE], fr, tag="obs", name=f"obs_s{s}")
                nc.sync.dma_start(t[:], obsTd[:, s * STAGE:(s + 1) * STAGE])
                stage_tiles[s] = t

            issue_stage(0)
            issue_stage(1)

            # const loads go on the vector DGE queue so they overlap the obs
            # stage loads on the sync queue
            wsb = const.tile([128, WCOLS], fr, tag="wsb", name="wsb")
            nc.scalar.dma_start(wsb[:], wpackd[:])
            wvsb = const.tile([128, 4], f32, tag="wvsb", name="wvsb")
            nc.scalar.dma_start(wvsb[:], wvd[:])
            brsb = const.tile([1, BRCOLS], fr, tag="brsb", name="brsb")
            nc.scalar.dma_start(brsb[:], browsd[:])
            csb = const.tile([128, 26], f32, tag="csb", name="csb")
            nc.scalar.dma_start(csb[:], cpackd[:])
            mask_sb = const.tile([128, ncols], f32, tag="mask", name="mask_sb")
            nc.scalar.dma_start(mask_sb[:], maskvd[:])

            # dedicated tiles per constant group: pool-tile APs compose
            # incorrectly when sliced twice, so every use below must be a
            # single-level slice of its own tile
            def cgroup(c0, c1, tg):
                t = const.tile([128, c1 - c0], f32, tag=tg, name=tg)
                nc.vector.tensor_copy(t[:], csb[:, c0:c1])
                return t
            b1c = cgroup(0, 2, "b1t")
            bgc = cgroup(2, 4, "bgt")
            lnwc = cgroup(4, 6, "lnwt")
            lnbc = cgroup(6, 8, "lnbt")
            wgatec = cgroup(8, 10, "wgt")
            wglnc = cgroup(10, 12, "wglnt")
            bdc = cgroup(12, 14, "bdt")
            bp1c = cgroup(14, 18, "bp1t")
            bp2c = cgroup(18, 22, "bp2t")
            bgate_ap = csb[0:1, 22:23]
            bv_ap = csb[0:1, 23:24]
            a0pb_ap = csb[0:1, 24:25]   # Wgate.ln_b + bgate
            na1_ap = csb[0:1, 25:26]    # -(Wgate.ln_w)

            def wslice(c0, c1):
                return wsb[:, c0:c1]

            def w2slice(c0, c1):
                return w2sb[:, c0:c1]

            # gate lhsT: [wgln_k | ones] per k-block, bf16
            wg2 = const.tile([128, 4], bf16, tag="wg2", name="wg2")
            nc.vector.tensor_copy(wg2[:, 0:1], wglnc[:, 0:1])
            nc.vector.tensor_copy(wg2[:, 2:3], wglnc[:, 1:2])
            wg2f = const.tile([128, 2], f32, tag="wg2f", name="wg2f")
            nc.vector.memset(wg2f[:], 1.0)
            nc.vector.tensor_copy(wg2[:, 1:2], wg2f[:, 0:1])
            nc.vector.tensor_copy(wg2[:, 3:4], wg2f[:, 1:2])

            ones_col_f = const.tile([1, 128], f32, tag="ones_col_f")
            nc.vector.memset(ones_col_f[:], 1.0)
            ones128_f = const.tile([128, 1], f32, tag="ones128_f")
            nc.vector.memset(ones128_f[:], 1.0)

            # resident Xg (bf16), per hid-block
            xg0 = const.tile([128, n_sh], bf16, tag="xg0", name="xg0")
            xg1 = const.tile([128, n_sh], bf16, tag="xg1", name="xg1")
            xg_m = [xg0, xg1]

            sq_acc = const.tile([128, 2 * n_chunks], f32, tag="sq_acc")
            pool_acc = [const.tile([128, n_blk], f32, tag=f"pool_acc{m}",
                                   name=f"pool_acc{m}") for m in range(2)]
            # gate logits (row 0) + per-node colsums (row 1), chunk-ordered
            # along free; one copy per TWO chunks via a double-bank PSUM tile
            glr2 = const.tile([2, n_sh], bf16, tag="glr2", name="glr2")
            glx = const.tile([n_chunks, CH], bf16, tag="glx", name="glx")
            gsx = const.tile([n_chunks, CH], bf16, tag="gsx", name="gsx")

            # ---- Phase A: pipelined mm1/mm2/gate ----
            px_t = {}
            xt_t = {}

            def emit_mm1(c):
                s = c // cps
                rhs = stage_tiles[s][:, (c % cps) * CH:(c % cps + 1) * CH]
                for m in range(2):
                    px = ps_x.tile([128, CH], f32, tag="px", name=f"px_{c}_{m}")
                    nc.tensor.matmul(px[:],
                                     wslice(W1_O + m * 128, W1_O + (m + 1) * 128),
                                     rhs, start=True, stop=True)
                    px_t[(c, m)] = px

            def emit_xt(c):
                # k=0 on scalar, k=1 on vector
                t0 = xt_p.tile([128, CH], fr, tag="xt", name=f"xt_{c}_0")
                nc.scalar.activation(t0[:], px_t[(c, 0)][:], AF.Relu,
                                     bias=b1c[:, 0:1])
                t1 = xt_p.tile([128, CH], fr, tag="xt", name=f"xt_{c}_1")
                nc.vector.tensor_scalar(t1[:], px_t[(c, 1)][:], b1c[:, 1:2], 0.0,
                                        OP.add, OP.max)
                if dbg and c == 0:
                    nc.sync.dma_start(dbg_b1t[:], b1c[:])
                    nc.sync.dma_start(dbg_csb[:], csb[:])
                    pxc = const.tile([128, CH], f32, tag="dbpx", name="dbpx")
                    nc.vector.tensor_copy(pxc[:], px_t[(c, 0)][:])
                    nc.sync.dma_start(dbg_px0[:], pxc[:])
                    nc.sync.dma_start(dbg_obs0[:],
                                      stage_tiles[0][:, 0:CH])
                    nc.sync.dma_start(dbg_xt0[:], t0[:])
                    nc.sync.dma_start(dbg_xt1[:], t1[:])
                xt_t[(c, 0)] = t0
                xt_t[(c, 1)] = t1
                del px_t[(c, 0)], px_t[(c, 1)]

            def emit_mm2(c):
                sl = slice(c * CH, (c + 1) * CH)
                for m in range(2):
                    pxg = ps_xg.tile([128, CH], f32, tag="pxg", name=f"pxg_{c}_{m}")
                    for k in range(2):
                        lo = WG_O + k * 256 + m * 128
                        nc.tensor.matmul(pxg[:], wslice(lo, lo + 128),
                                         xt_t[(c, k)][:],
                                         start=(k == 0), stop=(k == 1))
                    nc.scalar.activation(xg_m[m][:, sl], pxg[:], AF.Relu,
                                         bias=bgc[:, m:m + 1])
                del xt_t[(c, 0)], xt_t[(c, 1)]

            def emit_sq(c):
                sl = slice(c * CH, (c + 1) * CH)
                for m in range(2):
                    scr = scrq_p.tile([128, CH], bf16, tag="scr",
                                      name=f"sq_{c}_{m}")
                    nc.vector.scalar_tensor_tensor(
                        scr[:], xg_m[m][:, sl], 1.0, xg_m[m][:, sl],
                        OP.mult, OP.mult,
                        accum_out=sq_acc[:, 2 * c + m:2 * c + m + 1])

            gate_grp = {}

            def emit_gate(c):
                sl = slice(c * CH, (c + 1) * CH)
                p, r = divmod(c, 2)
                if r == 0:
                    gate_grp[p] = ps_gl.tile([2, 2 * CH], f32, tag="pgl",
                                             name=f"pgl_p{p}")
                out = gate_grp[p][0:2, r * CH:(r + 1) * CH]
                nc.tensor.matmul(out, wg2[:, 0:2], xg_m[0][:, sl],
                                 start=True, stop=False)
                nc.tensor.matmul(out, wg2[:, 2:4], xg_m[1][:, sl],
                                 start=False, stop=True)
                if r == 1:
                    sl2 = slice(p * 2 * CH, (p + 1) * 2 * CH)
                    nc.vector.tensor_copy(glr2[:, sl2], gate_grp[p][:])
                    del gate_grp[p]

            emit_mm1(0)
            emit_xt(0)
            for c in range(n_chunks):
                if c % cps == 0 and c // cps >= 1 and c // cps + 1 < n_stages:
                    issue_stage(c // cps + 1)
                emit_mm2(c)
                if c + 1 < n_chunks:
                    emit_mm1(c + 1)
                    emit_xt(c + 1)
                if c >= 1:
                    emit_gate(c - 1)
                    emit_sq(c - 1)
            emit_gate(n_chunks - 1)
            emit_sq(n_chunks - 1)
            # preload the sqrt/sigmoid act table while waiting on the
            # stats AllReduce (table load otherwise lands on the critical
            # path right after it)
            tblw = sm_p.tile([1, 1], f32, tag="tblw")
            nc.scalar.activation(tblw[:], ones_col_f[0:1, 0:1], AF.Sqrt)

            # ---- Phase B: global LN stats + AllReduce ----
            nc.sync.dma_start(glx[:], glr2[0:1, :])
            nc.sync.dma_start(gsx[:], glr2[1:2, :])
            su_red = sm_p.tile([n_chunks, 1], f32, tag="su_red")
            nc.vector.tensor_reduce(su_red[:], gsx[:], AX.X, OP.add)
            sq_red = sm_p.tile([128, 1], f32, tag="sq_red")
            nc.vector.tensor_reduce(sq_red[:], sq_acc[:], AX.X, OP.add)
            ps_st = ps_x.tile([1, 2], f32, tag="px", name="ps_st")
            nc.tensor.matmul(ps_st[0:1, 0:1], su_red[:], ones128_f[0:n_chunks, :],
                             start=True, stop=True)
            nc.tensor.matmul(ps_st[0:1, 1:2], sq_red[:], ones128_f[:],
                             start=True, stop=True)
            stats_sb = sm_p.tile([1, 2], f32, tag="stats")
            nc.vector.tensor_copy(stats_sb[:], ps_st[:])

            st_in = dram.tile([1, 2], f32, tag="st_in")
            st_out = dram.tile([1, 2], f32, tag="st_out")
            nc.gpsimd.dma_start(st_in[:], stats_sb[:])
            nc.gpsimd.collective_compute(
                "AllReduce", OP.add, replica_groups=rg,
                ins=[st_in.opt()], outs=[st_out.opt()])
            stats_g = sm_p.tile([1, 2], f32, tag="stats_g")
            nc.gpsimd.dma_start(stats_g[:], st_out[:])

            ps_b = ps_gl.tile([128, 2], f32, tag="pgl", name="ps_b")
            nc.tensor.matmul(ps_b[:], ones_col_f[:], stats_g[:], start=True, stop=True)
            stats_bc = sm_p.tile([128, 2], f32, tag="stats_bc")
            nc.vector.tensor_copy(stats_bc[:], ps_b[:])

            mu = sm_p.tile([128, 1], f32, tag="mu")
            nc.vector.tensor_scalar(mu[:], stats_bc[:, 0:1], 1.0 / MTOT, None, OP.mult)
            e2 = sm_p.tile([128, 1], f32, tag="e2")
            nc.vector.tensor_scalar(e2[:], stats_bc[:, 1:2], 1.0 / MTOT, None, OP.mult)
            var = sm_p.tile([128, 1], f32, tag="var")
            nc.vector.scalar_tensor_tensor(var[:], mu[:], mu[:, 0:1], e2[:],
                                           OP.mult, OP.subtract)
            nc.vector.tensor_scalar(var[:], var[:], -1.0, None, OP.mult)
            sd = sm_p.tile([128, 1], f32, tag="sd")
            nc.scalar.activation(sd[:], var[:], AF.Sqrt)
            # eps=1e-5 vs sd~0.6: skipping the +eps shifts inv by ~2e-5 rel
            inv = sm_p.tile([128, 1], f32, tag="inv")
            nc.vector.reciprocal(inv[:], sd[:])

            # gate_const = (Wgate.ln_b + bgate) - (Wgate.ln_w) * mu * inv
            muinv = sm_p.tile([1, 1], f32, tag="muinv")
            nc.vector.tensor_tensor(muinv[:], mu[0:1, :], inv[0:1, :], OP.mult)
            gconst = sm_p.tile([1, 1], f32, tag="gconst")
            nc.vector.tensor_scalar(gconst[:], muinv[:], na1_ap, a0pb_ap,
                                    OP.mult, OP.add)
            ps_g32 = ps_x.tile([n_chunks, 1], f32, tag="px", name="ps_g32")
            nc.tensor.matmul(ps_g32[:], ones_col_f[0:1, 0:n_chunks],
                             gconst[:], start=True, stop=True)
            gc32 = sm_p.tile([n_chunks, 1], f32, tag="gc32")
            nc.vector.tensor_copy(gc32[:], ps_g32[:])
            invr = sm_p.tile([1, 1], f32, tag="invr")
            nc.vector.tensor_copy(invr[:], inv[0:1, :])
            ps_i32 = ps_x.tile([n_chunks, 1], f32, tag="px", name="ps_i32")
            nc.tensor.matmul(ps_i32[:], ones_col_f[0:1, 0:n_chunks],
                             invr[:], start=True, stop=True)
            inv32 = sm_p.tile([n_chunks, 1], f32, tag="inv32")
            nc.vector.tensor_copy(inv32[:], ps_i32[:])

            # ---- Phase C: gate = sigmoid(inv*logit + const); gated pooling ----
            gate_sb = const.tile([n_chunks, CH], bf16, tag="gate_sb")
            gs_part = sm_p.tile([n_chunks, 1], f32, tag="gs_part")
            nc.scalar.activation(gate_sb[:], glx[:], AF.Sigmoid,
                                 bias=gc32[:], scale=inv32[:],
                                 accum_out=gs_part[:])
            ps_gs = ps_x.tile([1, 1], f32, tag="px", name="ps_gs")
            nc.tensor.matmul(ps_gs[:], gs_part[:], ones128_f[0:n_chunks, :],
                             start=True, stop=True)
            gsum_sb = sm_p.tile([1, 1], f32, tag="gsum")
            nc.vector.tensor_copy(gsum_sb[:], ps_gs[:])

            # gate rows -> [1, n_sh] in DRAM, then stride-0 broadcast reads
            # (SBUF partition dims need nonzero step; DRAM APs are linear)
            g_rowd = dram.tile([1, n_sh], bf16, tag="g_rowd", name="g_rowd")
            nc.sync.dma_start(g_rowd[:], gate_sb[:])

            # pooling: TENSOR_TENSOR multiply runs at the 2x bf16 DVE rate
            # (STT does not on this hw); accumulation via scalar Copy-
            # activation accum (scalar is idle here), with a few direct STTs
            # interleaved to balance the two engines
            last_grep = None
            qeng = [nc.sync, nc.scalar, nc.gpsimd]
            for b in range(n_blk):
                g_rep = grep_p.tile([128, BLK], bf16, tag="grep", name=f"grep_{b}")
                if b == n_blk - 1:
                    last_grep = g_rep
                base = g_rowd[0:1, b * BLK:(b + 1) * BLK]
                src = bass.AP(base.tensor, base.offset, [[0, 128], [1, BLK]])
                qeng[b % 3].dma_start(g_rep[:], src)
                for m in range(2):
                    j = 2 * b + m
                    xga = xg_m[m][:, b * BLK:(b + 1) * BLK]
                    if j % 3 == 2:
                        scr = scrp_p.tile([128, BLK], bf16, tag="scr",
                                          name=f"pl_{b}_{m}")
                        nc.vector.scalar_tensor_tensor(
                            scr[:], g_rep[:], 1.0, xga, OP.mult, OP.mult,
                            accum_out=pool_acc[m][:, b:b + 1])
                    else:
                        scr = scrp_p.tile([128, BLK], bf16, tag="scr",
                                          name=f"pl_{b}_{m}")
                        nc.vector.tensor_tensor(scr[:], g_rep[:], xga, OP.mult)
                        scrc = scrp_p.tile([128, BLK], bf16, tag="scrc",
                                           name=f"plc_{b}_{m}")
                        nc.scalar.activation(scrc[:], scr[:], AF.Copy,
                                             accum_out=pool_acc[m][:, b:b + 1])

            # ---- Phase D: pooled AllReduce + affine ----
            # scale2/shift2 here: off the AR1->sigmoid critical path, ready
            # before the AR2 result lands
            scale2 = sm_p.tile([128, 2], f32, tag="scale2")
            nc.vector.tensor_scalar(scale2[:], lnwc[:], inv[:], None, OP.mult)
            mscale = sm_p.tile([128, 2], f32, tag="mscale")
            nc.vector.tensor_scalar(mscale[:], scale2[:], mu[:], None, OP.mult)
            shift2 = sm_p.tile([128, 2], f32, tag="shift2")
            nc.vector.tensor_tensor(shift2[:], lnbc[:], mscale[:], OP.subtract)
            # gsum already broadcast across partitions pre-AR (col 2)
            ps_gb = ps_gl.tile([128, 1], f32, tag="pgl", name="ps_gb")
            nc.tensor.matmul(ps_gb[:], ones_col_f[:], gsum_sb[:],
                             start=True, stop=True)
            pack = sm_p.tile([128, 3], f32, tag="pack")
            nc.vector.tensor_reduce(pack[:, 0:1], pool_acc[0][:], AX.X, OP.add)
            nc.vector.tensor_reduce(pack[:, 1:2], pool_acc[1][:], AX.X, OP.add)
            nc.vector.tensor_copy(pack[:, 2:3], ps_gb[:])

            pk_in = dram.tile([128, 3], f32, tag="pk_in")
            pk_out = dram.tile([128, 3], f32, tag="pk_out")
            nc.gpsimd.dma_start(pk_in[:], pack[:])
            nc.gpsimd.collective_compute(
                "AllReduce", OP.add, replica_groups=rg,
                ins=[pk_in.opt()], outs=[pk_out.opt()])
            arp = sm_p.tile([128, 3], f32, tag="arp")
            nc.gpsimd.dma_start(arp[:], pk_out[:])

            t1 = sm_p.tile([128, 2], f32, tag="t1")
            nc.vector.tensor_tensor(t1[:], scale2[:], arp[:, 0:2], OP.mult)
            t2 = sm_p.tile([128, 2], f32, tag="t2")
            nc.vector.tensor_scalar(t2[:], shift2[:], arp[:, 2:3], None, OP.mult)
            poolc = sm_p.tile([128, 2], fr, tag="poolc")
            nc.vector.tensor_tensor(poolc[:], t1[:], t2[:], OP.add)

            # ---- Phase E: MLP, row-oriented (fr weights stream at 1 cyc/row,
            # h-column lhsT keeps LDWEIGHTS tiny; PE transposes restore
            # column form between layers) ----
            ident1 = const.tile([1, 1], f32, tag="ident1")
            nc.vector.memset(ident1[:], 1.0)
            one_fr = const.tile([1, 1], fr, tag="one_fr")
            nc.vector.tensor_copy(one_fr[:], ident1[:])

            def row_layer(h_cols, nk, w_off, w_width, nout, b_off, tag):
                ps = ps_x.tile([1, nout], f32, tag="px", name=f"ps_{tag}")
                for k in range(nk):
                    lo = w_off + k * w_width
                    nc.tensor.matmul(ps[:], h_cols[:, k:k + 1],
                                     wslice(lo, lo + w_width),
                                     start=(k == 0), stop=False)
                nc.tensor.matmul(ps[:], one_fr[:],
                                 brsb[0:1, b_off:b_off + nout],
                                 start=False, stop=True)
                h_row = sm_p.tile([1, nout], f32, tag=f"hr_{tag}",
                                  name=f"hr_{tag}")
                nc.scalar.activation(h_row[:], ps[:], AF.Relu)
                nko = nout // 128
                hc = sm_p.tile([128, nko], fr, tag=f"hc_{tag}", name=f"hc_{tag}")
                for k in range(nko):
                    # psxg ring (3 bufs) lets transposes run back-to-back on
                    # the PE while the copies pipeline on the vector engine
                    ps_t = ps_xg.tile([128, 1], f32, tag="pxg",
                                      name=f"pt_{tag}{k}")
                    nc.tensor.transpose(ps_t[:], h_row[0:1, k * 128:(k + 1) * 128],
                                        ident1[:])
                    nc.vector.tensor_copy(hc[:, k:k + 1], ps_t[:])
                return hc

            # poolc must be fr for the row matmuls
            h1c = row_layer(poolc, 2, WD_O, 256, HID, BD_O, "h1")
            h2c = row_layer(h1c, 2, WP1_O, 512, POL, BP1_O, "h2")
            h3c = row_layer(h2c, 4, WP2_O, 512, POL, BP2_O, "h3")
            # final: value = h3 . Wv + bv  (f32: fp32r rejects N=1)
            h3f = sm_p.tile([128, 4], f32, tag="h3f")
            nc.vector.tensor_copy(h3f[:], h3c[:])
            psv = ps_gl.tile([1, 1], f32, tag="pgl", name="psv")
            for k in range(4):
                nc.tensor.matmul(psv[:], h3f[:, k:k + 1], wvsb[:, k:k + 1],
                                 start=(k == 0), stop=False)
            # + bv via a K=1 accumulating matmul: stays on the PE queue,
            # no cross-engine hop
            nc.tensor.matmul(psv[:], ident1[:], bv_ap, start=False, stop=True)
            val_sb = sm_p.tile([1, 1], f32, tag="val_sb")
            nc.vector.tensor_copy(val_sb[:], psv[:])

            ps_v = ps_gl.tile([128, 1], f32, tag="pgl", name="ps_v")
            nc.tensor.matmul(ps_v[:], ones_col_f[:], val_sb[:], start=True, stop=True)

            if dbg:
                def dump(dramp, ap):
                    nc.sync.dma_start(dramp[:], ap)
                dump(dbg_xg0, xg_m[0][:, 0:512])
                dump(dbg_glr2, glr2[:])
                dump(dbg_glx, glx[:])
                dump(dbg_gsx, gsx[:])
                dump(dbg_stats, stats_sb[:])
                dump(dbg_statsg, stats_g[:])
                dump(dbg_gate, gate_sb[:])
                dump(dbg_pack, pack[:])
                dump(dbg_arp, arp[:])
                dump(dbg_poolc, poolc[:])
                dump(dbg_h1, h1c[:])
                dump(dbg_grep, last_grep[:])

            outt = const.tile([128, ncols], f32, tag="outt")
            nc.vector.tensor_scalar(outt[:], mask_sb[:], ps_v[:, 0:1], None,
                                    OP.mult)
            nc.sync.dma_start(outd[:], outt[:])

    return nc


_NC_CACHE = {}


def _get_nc(n_sh, ncores, total_nodes):
    key = (n_sh, ncores, total_nodes)
    if key not in _NC_CACHE:
        _NC_CACHE[key] = build(n_sh, ncores, total_nodes)
    return _NC_CACHE[key]


def make_in_maps(observation, mask, W1, b1, Wg, bg, ln_w, ln_b, Wgate, bgate,
                 Wd, bd, Wp1, bp1, Wp2, bp2, Wv, bv,
                 n_sh=N_SH, ncores=NCORES):
    f = np.float32
    obs = np.asarray(observation, f)
    mask = np.asarray(mask, f).reshape(-1)

    def cols(v, nk):
        return np.asarray(v, f).reshape(nk, 128).T

    def blocks(W, fan_in):
        W = np.asarray(W, f)
        return [W[k * 128:(k + 1) * 128, :] for k in range(fan_in // 128)]

    wpack = np.concatenate(
        [np.asarray(W1, f)] + blocks(Wg, HID) + blocks(Wd, HID)
        + blocks(Wp1, HID) + blocks(Wp2, POL), axis=1)
    assert wpack.shape == (128, 4352), wpack.shape
    wv = np.concatenate(blocks(np.asarray(Wv, f).reshape(POL, 1), POL), axis=1)
    assert wv.shape == (128, 4), wv.shape
    brows = np.concatenate([np.asarray(bd, f).reshape(1, -1),
                            np.asarray(bp1, f).reshape(1, -1),
                            np.asarray(bp2, f).reshape(1, -1)], axis=1)
    assert brows.shape == (1, 1280), brows.shape

    wgln = np.asarray(Wgate, f).reshape(-1) * np.asarray(ln_w, f).reshape(-1)
    c22 = np.zeros((128, 1), f)
    c22[0, 0] = np.asarray(bgate, f).reshape(-1)[0]
    c23 = np.zeros((128, 1), f)
    c23[0, 0] = np.asarray(bv, f).reshape(-1)[0]
    wgv = np.asarray(Wgate, f).reshape(-1)
    c24 = np.zeros((128, 1), f)
    c24[0, 0] = float(wgv @ np.asarray(ln_b, f).reshape(-1)) \
        + float(np.asarray(bgate, f).reshape(-1)[0])
    c25 = np.zeros((128, 1), f)
    c25[0, 0] = -float(wgv @ np.asarray(ln_w, f).reshape(-1))
    cpack = np.concatenate(
        [cols(b1, 2), cols(bg, 2), cols(ln_w, 2), cols(ln_b, 2),
         cols(Wgate, 2), cols(wgln, 2), cols(bd, 2), cols(bp1, 4),
         cols(bp2, 4), c22, c23, c24, c25], axis=1)
    assert cpack.shape == (128, 26), cpack.shape

    shared = dict(
        wpack=np.ascontiguousarray(wpack),
        wv=np.ascontiguousarray(wv),
        brows=np.ascontiguousarray(brows),
        cpack=np.ascontiguousarray(cpack),
    )
    in_maps = []
    ncols = n_sh // 128
    for i in range(ncores):
        sl = slice(i * n_sh, (i + 1) * n_sh)
        in_maps.append(dict(
            obsT=np.ascontiguousarray(obs[sl].T),
            maskv=np.ascontiguousarray(mask[sl].reshape(128, ncols)),
            **shared,
        ))
    return in_maps


def kernel(observation, mask, edge_index, W1, b1, Wg, bg, ln_w, ln_b,
           Wgate, bgate, Wd, bd, Wp1, bp1, Wp2, bp2, Wv, bv):
    from concourse.bass_utils import run_bass_kernel_spmd

    nc = _get_nc(N_SH, NCORES, N_TOTAL)
    in_maps = make_in_maps(observation, mask, W1, b1, Wg, bg, ln_w, ln_b,
                           Wgate, bgate, Wd, bd, Wp1, bp1, Wp2, bp2, Wv, bv)
    res = run_bass_kernel_spmd(nc, in_maps, list(range(NCORES)))
    shards = [res.results[i]["out"].reshape(N_SH, 1) for i in range(NCORES)]
    return np.concatenate(shards, axis=0).astype(np.float32)
